# revision 4
# baseline (speedup 1.0000x reference)
"""Trainium2 Bass kernel for EntityAttention.

    beta[b,e,a] = (agent[b,e] @ w_psi) . (vis[b,e,a] @ w_phi)
    out         = softmax_a(beta)

Refactored so the huge `visible_observations` tensor is read exactly once,
in its natural layout, by a fused multiply+cumulative-sum on the Vector
engine (a custom DVE op: out = cumsum(in0 * in1)); per-a segment sums are
recovered by differencing the cumsum at segment boundaries:

    qT[k, be]   = sum_din w_psi[din, k] * agent[be, din]      (PE)
    t[be, dout] = sum_k   qT[k, be]     * w_phiT[k, dout]     (PE)
    cum         = cumsum_f(vis[be, (a,d)] * t[be, d bcast])   (DVE, 1 op / 8 a)
    beta[be, a] = cum[.., a*D+D-1] - cum[.., a*D-1]           (DVE, tiny)
    out[be, a]  = softmax_a(beta)                             (DVE + ACT)

Sharding: data-parallel over the batch axis across 8 NeuronCores
(16 batches / core); w_psi / w_phi replicated.

DMA: `visible` streams on the SP HWDGE queue; all small transfers
(weights, agent, outputs) go on the Activation HWDGE queue so they never
serialize in front of the 16 MB visible stream.
"""

from contextlib import ExitStack

import numpy as np

import concourse.bass as bass
import concourse.tile as tile
from concourse import bacc, bass_utils, dve_ops, mybir
from concourse.dve_spec import AluOp, Spec, Src0, Src1, _has_src1, lower, scan
from concourse.dve_uop import DveOpSpec
from concourse.masks import make_identity

# Problem shape (hardcoded per contract; kernel.py must be self-contained).
B, E, A, D, K = 128, 32, 16, 512, 128
N_CORES = 8
B_SH = B // N_CORES          # batches per core = 16
BE = B_SH * E                # rows per core = 512
NBC = BE // 128              # be-chunks of 128 partitions = 4
NDC = D // 128               # din-chunks = 4
HALF_A = 8                   # visible-agents per streamed half tile
F32 = mybir.dt.float32


# ---- custom DVE op: out = cumsum(in0 * in1) along the free axis ---------- #

def _ref_cumsum_mul(in0, in1, s0, s1, imm2):
    p = in0.shape[0]
    a = np.asarray(in0, np.float32).reshape(p, -1)
    b = np.ascontiguousarray(np.asarray(in1, np.float32)).reshape(p, -1)
    if b.shape[1] != a.shape[1]:
        b = np.tile(b, (1, a.shape[1] // b.shape[1]))
    return np.cumsum(a * b, axis=-1, dtype=np.float32)


def _register_cumsum_mul():
    name = "CUMSUM_MUL_ANT"
    if name in dve_ops._SUB_OPCODE_FOR_NAME:
        return next(op for op in dve_ops.OPS if op.name == name)
    spec = Spec(body=scan(AluOp.ADD, Src0 * Src1), reference=_ref_cumsum_mul)
    row = dve_ops._CUSTOM_DVE_ROW_BASE + len(dve_ops.OPS)
    assert row < 0x20
    shas = {}
    for ver in ("v3", "v4"):
        d = DveOpSpec(name=name, opcode=row, uops=lower(spec, ver=ver),
                      rd1_en=_has_src1(spec))
        shas[ver] = d.sha(ver)
    op = dve_ops.DveOp(name, spec, subdim=False, uops_sha=shas)
    dve_ops._SUB_OPCODE_FOR_NAME[name] = row
    dve_ops.OPS.append(op)
    dve_ops.CUSTOM_DVE_SPECS[name] = spec
    return op


CUMSUM_MUL = _register_cumsum_mul()


def _bcast_mid(ap_2d, count):
    """[P, N] AP -> [P, count, N] AP with a step-0 middle dim."""
    return bass.AP(
        tensor=ap_2d.tensor,
        offset=ap_2d.offset,
        ap=[ap_2d.ap[0], [0, count], *ap_2d.ap[1:]],
    )


def _emit(tc, nc, ag_d, vis_d, wpsi_d, wphi_d, out_d):
    with ExitStack() as ctx:
        const = ctx.enter_context(tc.tile_pool(name="const", bufs=1))
        agp = ctx.enter_context(tc.tile_pool(name="agp", bufs=2))
        visp = ctx.enter_context(tc.tile_pool(name="visp", bufs=3))
        cump = ctx.enter_context(tc.tile_pool(name="cump", bufs=2))
        small = ctx.enter_context(tc.tile_pool(name="small", bufs=4))
        ps_tr = ctx.enter_context(tc.tile_pool(name="ps_tr", bufs=3, space="PSUM"))
        ps_mm = ctx.enter_context(tc.tile_pool(name="ps_mm", bufs=2, space="PSUM"))

        ident = const.tile([128, 128], F32)
        make_identity(nc, ident)

        # Weights with interleaved din/dout chunking (chunk r = rows = r mod 4)
        # so each partition line is a contiguous 2 KB DMA.
        # w4[p, r, k] = w[4p + r, k]
        wpsi_sb = const.tile([128, NDC, K], F32)
        nc.scalar.dma_start(out=wpsi_sb, in_=wpsi_d.rearrange("(p r) k -> p r k", r=NDC))
        wphi_sb = const.tile([128, NDC, K], F32)
        nc.scalar.dma_start(out=wphi_sb, in_=wphi_d.rearrange("(p r) k -> p r k", r=NDC))

        # w_phiT with natural dout order: wphiT4[k, dl, r] = w_phi[4*dl + r, k],
        # flat free index f = dl*4 + r = dout.
        wphiT_sb = const.tile([128, 128, NDC], F32)
        for r in range(NDC):
            tr_ps = ps_tr.tile([128, 128], F32, tag="tr", name=f"trw{r}")
            nc.tensor.transpose(tr_ps, wphi_sb[:, r, :], ident)
            nc.scalar.copy(wphiT_sb[:, :, r], tr_ps)

        # agT[p, r, be] = agent[be, 4p + r]
        agT_sb = const.tile([128, NDC, BE], F32)
        qT_sb = const.tile([128, BE], F32)
        t_tiles = []
        for c in range(NBC):
            cs = slice(c * 128, (c + 1) * 128)
            ag_sb = agp.tile([128, D], F32, tag="ag", name=f"ag{c}")
            nc.scalar.dma_start(out=ag_sb, in_=ag_d[cs, :])
            ag_v = ag_sb.rearrange("p (q r) -> p q r", r=NDC)
            for r in range(NDC):
                tr_ps = ps_tr.tile([128, 128], F32, tag="tr", name=f"tra{c}_{r}")
                nc.tensor.transpose(tr_ps, ag_v[:, :, r], ident)
                nc.scalar.copy(agT_sb[:, r, cs], tr_ps)
            # qT[:, cs] = sum_r w_psi_chunk_r.T @ agT_chunk_r
            qt_ps = ps_mm.tile([128, 128], F32, tag="qt", name=f"qt{c}")
            for r in range(NDC):
                nc.tensor.matmul(
                    qt_ps,
                    lhsT=wpsi_sb[:, r, :],
                    rhs=agT_sb[:, r, cs],
                    start=(r == 0),
                    stop=(r == NDC - 1),
                )
            nc.scalar.copy(qT_sb[:, cs], qt_ps)
            # t[be_c, dout] = qT[:, cs].T @ w_phiT
            t_ps = ps_mm.tile([128, D], F32, tag="t", name=f"tps{c}")
            nc.tensor.matmul(
                t_ps, lhsT=qT_sb[:, cs], rhs=wphiT_sb[:, :, :], start=True, stop=True
            )
            t_sb = const.tile([128, D], F32, tag=f"t{c}", name=f"t{c}")
            nc.scalar.copy(t_sb, t_ps)
            t_tiles.append(t_sb)

        # Stream visible; fused multiply+cumsum; segment-diff into beta; softmax.
        for c in range(NBC):
            cs = slice(c * 128, (c + 1) * 128)
            beta_sb = small.tile([128, A], F32, tag="beta", name=f"beta{c}")
            for h in range(A // HALF_A):
                h8 = h * HALF_A
                vis_sb = visp.tile([128, HALF_A, D], F32, tag="vis", name=f"vis{c}_{h}")
                nc.sync.dma_start(
                    out=vis_sb,
                    in_=vis_d[cs, h8 * D:(h8 + HALF_A) * D],
                )
                cum = cump.tile([128, HALF_A, D], F32, tag="cum", name=f"cum{c}_{h}")
                nc.vector._custom_dve(
                    CUMSUM_MUL,
                    out=cum,
                    in0=vis_sb,
                    in1=_bcast_mid(t_tiles[c], HALF_A),
                )
                # boundary values cum[:, s, D-1] -> [128, HALF_A]
                bnd = cum[:, :, D - 1:D].rearrange("p s o -> p (s o)")
                nc.vector.tensor_copy(beta_sb[:, h8:h8 + 1], bnd[:, 0:1])
                nc.vector.tensor_sub(
                    beta_sb[:, h8 + 1:h8 + HALF_A], bnd[:, 1:HALF_A],
                    bnd[:, 0:HALF_A - 1],
                )
            negm = small.tile([128, 1], F32, tag="negm", name=f"negm{c}")
            nc.vector.tensor_reduce(
                negm, beta_sb, axis=mybir.AxisListType.X,
                op=mybir.AluOpType.max, negate=True,
            )
            prob = small.tile([128, A], F32, tag="prob", name=f"prob{c}")
            ssum = small.tile([128, 1], F32, tag="ssum", name=f"ssum{c}")
            nc.scalar.activation(
                prob, beta_sb, mybir.ActivationFunctionType.Exp,
                bias=negm, scale=1.0, accum_out=ssum,
            )
            rec = small.tile([128, 1], F32, tag="rec", name=f"rec{c}")
            nc.vector.reciprocal(rec, ssum)
            osb = small.tile([128, A], F32, tag="osb", name=f"osb{c}")
            nc.vector.tensor_scalar_mul(osb, prob, rec)
            nc.scalar.dma_start(out=out_d[cs, :], in_=osb)


def _build_program():
    nc = bacc.Bacc("TRN2", target_bir_lowering=False, debug=False)
    ag_d = nc.dram_tensor("agent", (BE, D), F32, kind="ExternalInput").ap()
    vis_d = nc.dram_tensor("vis", (BE, A * D), F32, kind="ExternalInput").ap()
    wpsi_d = nc.dram_tensor("w_psi", (D, K), F32, kind="ExternalInput").ap()
    wphi_d = nc.dram_tensor("w_phi", (D, K), F32, kind="ExternalInput").ap()
    out_d = nc.dram_tensor("out", (BE, A), F32, kind="ExternalOutput").ap()
    with tile.TileContext(nc) as tc:
        _emit(tc, nc, ag_d, vis_d, wpsi_d, wphi_d, out_d)
    nc.compile()
    return nc


_PROG = None


def _get_program():
    global _PROG
    if _PROG is None:
        _PROG = _build_program()
    return _PROG


def make_in_maps(agent_observation, visible_observations, w_psi, w_phi):
    agent = np.ascontiguousarray(np.asarray(agent_observation, np.float32)).reshape(B, E, D)
    vis = np.ascontiguousarray(np.asarray(visible_observations, np.float32)).reshape(B, E, A, D)
    wpsi = np.ascontiguousarray(np.asarray(w_psi, np.float32))
    wphi = np.ascontiguousarray(np.asarray(w_phi, np.float32))
    in_maps = []
    for ci in range(N_CORES):
        sl = slice(ci * B_SH, (ci + 1) * B_SH)
        in_maps.append({
            "agent": np.ascontiguousarray(agent[sl].reshape(BE, D)),
            "vis": np.ascontiguousarray(vis[sl].reshape(BE, A * D)),
            "w_psi": wpsi,
            "w_phi": wphi,
        })
    return in_maps


def run_sharded(in_maps, trace=False, **kwargs):
    nc = _get_program()
    return bass_utils.run_bass_kernel_spmd(
        nc, in_maps, core_ids=list(range(N_CORES)), trace=trace, **kwargs
    )


def kernel(agent_observation, visible_observations, w_psi, w_phi):
    in_maps = make_in_maps(agent_observation, visible_observations, w_psi, w_phi)
    res = run_sharded(in_maps)
    return np.concatenate(
        [r["out"].reshape(B_SH, E, A) for r in res.results], axis=0
    )


# revision 8
# speedup vs baseline: 1.1509x; 1.1509x over previous
"""Trainium2 Bass kernel for EntityAttention.

    beta[b,e,a] = (agent[b,e] @ w_psi) . (vis[b,e,a] @ w_phi)
    out         = softmax_a(beta)

Refactored so the huge `visible_observations` tensor is read exactly once,
in its natural layout, by a fused multiply+cumulative-sum on the Vector
engine (a custom DVE op: out = cumsum(in0 * in1)); per-a segment sums are
recovered by differencing the cumsum at segment boundaries:

    qT[k, be]   = sum_din w_psi[din, k] * agent[be, din]      (PE)
    t[be, dout] = sum_k   qT[k, be]     * w_phiT[k, dout]     (PE)
    cum         = cumsum_f(vis[be, (a,d)] * t[be, d bcast])   (DVE, 1 op / 8 a)
    beta[be, a] = cum[.., a*D+D-1] - cum[.., a*D-1]           (DVE, tiny)
    out[be, a]  = softmax_a(beta)                             (DVE + ACT)

Sharding: data-parallel over the batch axis across 8 NeuronCores
(16 batches / core); w_psi / w_phi replicated.

DMA: `visible` streams on the SP HWDGE queue; all small transfers
(weights, agent, outputs) go on the Activation HWDGE queue so they never
serialize in front of the 16 MB visible stream.
"""

from contextlib import ExitStack

import numpy as np

import concourse.bass as bass
import concourse.tile as tile
from concourse import bacc, bass_utils, dve_ops, mybir
from concourse.dve_spec import AluOp, Spec, Src0, Src1, _has_src1, lower, scan
from concourse.dve_uop import DveOpSpec
from concourse.masks import make_identity

# Problem shape (hardcoded per contract; kernel.py must be self-contained).
B, E, A, D, K = 128, 32, 16, 512, 128
N_CORES = 8
B_SH = B // N_CORES          # batches per core = 16
BE = B_SH * E                # rows per core = 512
NBC = BE // 128              # be-chunks of 128 partitions = 4
NDC = D // 128               # din-chunks = 4
HALF_A = 8                   # visible-agents per streamed half tile
F32 = mybir.dt.float32


# ---- custom DVE op: out = cumsum(in0 * in1) along the free axis ---------- #

def _ref_cumsum_mul(in0, in1, s0, s1, imm2):
    p = in0.shape[0]
    a = np.asarray(in0, np.float32).reshape(p, -1)
    b = np.ascontiguousarray(np.asarray(in1, np.float32)).reshape(p, -1)
    if b.shape[1] != a.shape[1]:
        b = np.tile(b, (1, a.shape[1] // b.shape[1]))
    return np.cumsum(a * b, axis=-1, dtype=np.float32)


def _register_cumsum_mul():
    name = "CUMSUM_MUL_ANT"
    if name in dve_ops._SUB_OPCODE_FOR_NAME:
        return next(op for op in dve_ops.OPS if op.name == name)
    spec = Spec(body=scan(AluOp.ADD, Src0 * Src1), reference=_ref_cumsum_mul)
    row = dve_ops._CUSTOM_DVE_ROW_BASE + len(dve_ops.OPS)
    assert row < 0x20
    shas = {}
    for ver in ("v3", "v4"):
        d = DveOpSpec(name=name, opcode=row, uops=lower(spec, ver=ver),
                      rd1_en=_has_src1(spec))
        shas[ver] = d.sha(ver)
    op = dve_ops.DveOp(name, spec, subdim=False, uops_sha=shas)
    dve_ops._SUB_OPCODE_FOR_NAME[name] = row
    dve_ops.OPS.append(op)
    dve_ops.CUSTOM_DVE_SPECS[name] = spec
    return op


CUMSUM_MUL = _register_cumsum_mul()


def _bcast_mid(ap_2d, count):
    """[P, N] AP -> [P, count, N] AP with a step-0 middle dim."""
    return bass.AP(
        tensor=ap_2d.tensor,
        offset=ap_2d.offset,
        ap=[ap_2d.ap[0], [0, count], *ap_2d.ap[1:]],
    )


def _emit(tc, nc, ag_d, vis_d, wpsi_d, wphi_d, out_d):
    with ExitStack() as ctx:
        const = ctx.enter_context(tc.tile_pool(name="const", bufs=1))
        agp = ctx.enter_context(tc.tile_pool(name="agp", bufs=2))
        visp = ctx.enter_context(tc.tile_pool(name="visp", bufs=4))
        small = ctx.enter_context(tc.tile_pool(name="small", bufs=4))
        ps_tr = ctx.enter_context(tc.tile_pool(name="ps_tr", bufs=3, space="PSUM"))
        ps_mm = ctx.enter_context(tc.tile_pool(name="ps_mm", bufs=2, space="PSUM"))

        ident = const.tile([128, 128], F32)
        make_identity(nc, ident)

        # Weights with interleaved din/dout chunking (chunk r = rows = r mod 4)
        # so each partition line is a contiguous 2 KB DMA.
        # w4[p, r, k] = w[4p + r, k]
        wpsi_sb = const.tile([128, NDC, K], F32)
        nc.sync.dma_start(out=wpsi_sb, in_=wpsi_d.rearrange("(p r) k -> p r k", r=NDC))
        wphi_sb = const.tile([128, NDC, K], F32)
        nc.sync.dma_start(out=wphi_sb, in_=wphi_d.rearrange("(p r) k -> p r k", r=NDC))

        # w_phiT with natural dout order: wphiT4[k, dl, r] = w_phi[4*dl + r, k],
        # flat free index f = dl*4 + r = dout.
        wphiT_sb = const.tile([128, 128, NDC], F32)
        for r in range(NDC):
            tr_ps = ps_tr.tile([128, 128], F32, tag="tr", name=f"trw{r}")
            nc.tensor.transpose(tr_ps, wphi_sb[:, r, :], ident)
            nc.scalar.copy(wphiT_sb[:, :, r], tr_ps)

        # agT[p, r, be] = agent[be, 4p + r]
        agT_sb = const.tile([128, NDC, BE], F32)
        qT_sb = const.tile([128, BE], F32)
        t_tiles = []
        for c in range(NBC):
            cs = slice(c * 128, (c + 1) * 128)
            ag_sb = agp.tile([128, D], F32, tag="ag", name=f"ag{c}")
            nc.sync.dma_start(out=ag_sb, in_=ag_d[cs, :])
            ag_v = ag_sb.rearrange("p (q r) -> p q r", r=NDC)
            for r in range(NDC):
                tr_ps = ps_tr.tile([128, 128], F32, tag="tr", name=f"tra{c}_{r}")
                nc.tensor.transpose(tr_ps, ag_v[:, :, r], ident)
                nc.scalar.copy(agT_sb[:, r, cs], tr_ps)
            # qT[:, cs] = sum_r w_psi_chunk_r.T @ agT_chunk_r
            qt_ps = ps_mm.tile([128, 128], F32, tag="qt", name=f"qt{c}")
            for r in range(NDC):
                nc.tensor.matmul(
                    qt_ps,
                    lhsT=wpsi_sb[:, r, :],
                    rhs=agT_sb[:, r, cs],
                    start=(r == 0),
                    stop=(r == NDC - 1),
                )
            nc.scalar.copy(qT_sb[:, cs], qt_ps)
            # t[be_c, dout] = qT[:, cs].T @ w_phiT
            t_ps = ps_mm.tile([128, D], F32, tag="t", name=f"tps{c}")
            nc.tensor.matmul(
                t_ps, lhsT=qT_sb[:, cs], rhs=wphiT_sb[:, :, :], start=True, stop=True
            )
            t_sb = const.tile([128, D], F32, tag=f"t{c}", name=f"t{c}")
            nc.scalar.copy(t_sb, t_ps)
            t_tiles.append(t_sb)

        # Stream visible; fused multiply+cumsum (in place); segment-diff into
        # beta; softmax. The last chunk uses finer tiles to shrink the tail.
        for c in range(NBC):
            cs = slice(c * 128, (c + 1) * 128)
            beta_sb = small.tile([128, A], F32, tag="beta", name=f"beta{c}")
            groups = [8, 8] if c < NBC - 1 else [4, 4, 4, 4]
            a0 = 0
            for gi, na in enumerate(groups):
                vis_sb = visp.tile([128, HALF_A, D], F32, tag="vis",
                                   name=f"vis{c}_{gi}")[:, :na, :]
                nc.sync.dma_start(
                    out=vis_sb,
                    in_=vis_d[cs, a0 * D:(a0 + na) * D],
                )
                nc.vector._custom_dve(
                    CUMSUM_MUL,
                    out=vis_sb,
                    in0=vis_sb,
                    in1=_bcast_mid(t_tiles[c], na),
                )
                # boundary values cum[:, s, D-1] -> [128, na]
                bnd = vis_sb[:, :, D - 1:D].rearrange("p s o -> p (s o)")
                nc.vector.tensor_copy(beta_sb[:, a0:a0 + 1], bnd[:, 0:1])
                nc.vector.tensor_sub(
                    beta_sb[:, a0 + 1:a0 + na], bnd[:, 1:na], bnd[:, 0:na - 1],
                )
                a0 += na
            negm = small.tile([128, 1], F32, tag="negm", name=f"negm{c}")
            nc.vector.tensor_reduce(
                negm, beta_sb, axis=mybir.AxisListType.X,
                op=mybir.AluOpType.max, negate=True,
            )
            prob = small.tile([128, A], F32, tag="prob", name=f"prob{c}")
            ssum = small.tile([128, 1], F32, tag="ssum", name=f"ssum{c}")
            nc.scalar.activation(
                prob, beta_sb, mybir.ActivationFunctionType.Exp,
                bias=negm, scale=1.0, accum_out=ssum,
            )
            rec = small.tile([128, 1], F32, tag="rec", name=f"rec{c}")
            nc.vector.reciprocal(rec, ssum)
            osb = small.tile([128, A], F32, tag="osb", name=f"osb{c}")
            nc.vector.tensor_scalar_mul(osb, prob, rec)
            nc.scalar.dma_start(out=out_d[cs, :], in_=osb)


def _build_program():
    nc = bacc.Bacc("TRN2", target_bir_lowering=False, debug=False)
    ag_d = nc.dram_tensor("agent", (BE, D), F32, kind="ExternalInput").ap()
    vis_d = nc.dram_tensor("vis", (BE, A * D), F32, kind="ExternalInput").ap()
    wpsi_d = nc.dram_tensor("w_psi", (D, K), F32, kind="ExternalInput").ap()
    wphi_d = nc.dram_tensor("w_phi", (D, K), F32, kind="ExternalInput").ap()
    out_d = nc.dram_tensor("out", (BE, A), F32, kind="ExternalOutput").ap()
    with tile.TileContext(nc) as tc:
        _emit(tc, nc, ag_d, vis_d, wpsi_d, wphi_d, out_d)
    nc.compile()
    return nc


_PROG = None


def _get_program():
    global _PROG
    if _PROG is None:
        _PROG = _build_program()
    return _PROG


def make_in_maps(agent_observation, visible_observations, w_psi, w_phi):
    agent = np.ascontiguousarray(np.asarray(agent_observation, np.float32)).reshape(B, E, D)
    vis = np.ascontiguousarray(np.asarray(visible_observations, np.float32)).reshape(B, E, A, D)
    wpsi = np.ascontiguousarray(np.asarray(w_psi, np.float32))
    wphi = np.ascontiguousarray(np.asarray(w_phi, np.float32))
    in_maps = []
    for ci in range(N_CORES):
        sl = slice(ci * B_SH, (ci + 1) * B_SH)
        in_maps.append({
            "agent": np.ascontiguousarray(agent[sl].reshape(BE, D)),
            "vis": np.ascontiguousarray(vis[sl].reshape(BE, A * D)),
            "w_psi": wpsi,
            "w_phi": wphi,
        })
    return in_maps


def run_sharded(in_maps, trace=False, **kwargs):
    nc = _get_program()
    return bass_utils.run_bass_kernel_spmd(
        nc, in_maps, core_ids=list(range(N_CORES)), trace=trace, **kwargs
    )


def kernel(agent_observation, visible_observations, w_psi, w_phi):
    in_maps = make_in_maps(agent_observation, visible_observations, w_psi, w_phi)
    res = run_sharded(in_maps)
    return np.concatenate(
        [r["out"].reshape(B_SH, E, A) for r in res.results], axis=0
    )
